# revision 50
# baseline (speedup 1.0000x reference)
"""KoLeo loss kernel for Trainium2 (8 NeuronCores, SPMD row-sharded), fp8 edition.

Algorithm (matches the jax reference):
  feats_n = features / ||features||_row          (L2 row normalize)
  C       = feats_n @ feats_n.T                  (cosine similarity, NxN)
  m_i     = max_{j != i} C[i, j]                 (nearest-neighbor cosine)
  dist_i  = sqrt(2 - 2 m_i)                      (= ||f_i - f_j*|| for unit vectors)
  loss    = -mean(log(dists_i + 1e-8))

Distribution follows the torch/dist decomposition from the sharding hint:
each device L2-normalizes its OWN 2048-row shard on-chip, all-gathers the
normalized features (host plays the interconnect for the gather, staging the
quantized fp8 gather result in each core's DRAM), computes its 2048 x 16384
slice of the similarity against the gathered set, reduces a per-row max, and
the tiny final sqrt/log/mean runs on host.

Device pipeline per core (SPMD, identical program):
  - Inputs are rotated so the core's own shard is block 0; the C diagonal
    then sits in block 0 at fixed positions (16 constant masked adds).
  - Shard prep per row-tile: ScalarE Square+accum -> DVE reciprocal ->
    ScalarE Sqrt(scale) -> ScalarE normalize (fp8 -> bf16), bf16 DMA-xbar
    transpose, ScalarE cast-copy into the fp8 PLANE-separated layout that
    LdWeights dual-fp8 mode requires for the stationary operand.
  - Gathered blocks 1..7 ride the DMA xbar straight out of DRAM with
    consecutive fp8 pairs packed as uint16 - the pair-interleaved layout is
    accepted for the MOVING operand of DoubleRow matmuls.
  - Matmuls run fp8 DoubleRow (2 PE rows/cycle, K=256 per instruction):
    4x fewer PE cycles than bf16.
  - Row-max drain alternates blocks: odd blocks are staged PSUM->bf16 SBUF
    by ScalarE; even blocks drain on DVE via ONE tensor_tensor_reduce that
    maxes the live PSUM block against the staged block and reduces - each
    DVE pass retires two blocks (the ISA allows only one PSUM operand per
    instruction, so this is the cheapest legal drain).
"""

import os

import numpy as np

_KV = os.environ.get("KV", "")  # debug feature flags, comma separated

P = 128          # SBUF partitions
N_FULL = 16384
D_FULL = 1024
NCORES = 8
SCALE = 16.0     # fp8 range scale for normalized rows; dots come out x SCALE^2
BIG = -1024.0    # diagonal mask add (dots are ~ +-256 after scaling)


def _build(N, D, NC):
    import concourse.bacc as bacc
    import concourse.mybir as mybir
    from concourse import tile

    from concourse import masks

    f32 = mybir.dt.float32
    bf16 = mybir.dt.bfloat16
    fp8 = mybir.dt.float8e4
    u16 = mybir.dt.uint16
    AF = mybir.ActivationFunctionType
    DR = mybir.MatmulPerfMode.DoubleRow
    MAX = mybir.AluOpType.max

    SH = N // NC          # shard rows per core (2048)
    JB = SH               # column-block size
    nJ = N // JB          # column blocks (8)
    nR = JB // P          # row tiles per block (16)
    nI = nR               # row tiles in shard (16)
    KC = D // 256         # 256-deep DoubleRow super-chunks (4)
    C2 = 256              # fp8 elems per (kc, row-tile) = 2 x 128
    NCH = 512             # psum chunk cols
    LG = 2                # row-tiles per shard DMA load

    nc = bacc.Bacc("TRN2", target_bir_lowering=False, debug=False)
    # all-gathered normalized features (fp8, x SCALE), rotated shard-first
    gath = nc.dram_tensor("gathered", [N, D], fp8, kind="ExternalInput").ap()
    # raw (unnormalized) fp8 shard rows
    shard = nc.dram_tensor("shard", [SH, D], fp8, kind="ExternalInput").ap()
    outd = nc.dram_tensor("maxdot", [SH], f32, kind="ExternalOutput").ap()

    with tile.TileContext(nc) as tc:
        with (
            tc.tile_pool(name="const", bufs=1) as constp,
            tc.tile_pool(name="at", bufs=1) as atp,
            tc.tile_pool(name="bt", bufs=3) as btp,
            tc.tile_pool(name="x", bufs=8) as xp,
            tc.tile_pool(name="sq", bufs=4) as sqp,
            tc.tile_pool(name="ss", bufs=1) as ssp,
            tc.tile_pool(name="xn", bufs=6) as xnp,
            tc.tile_pool(name="tstg", bufs=6) as tstgp,
            tc.tile_pool(name="stg", bufs=3) as stgp,
            tc.tile_pool(name="sc", bufs=3) as scp,
            tc.tile_pool(name="macc", bufs=1) as maccp,
            tc.tile_pool(name="acc", bufs=4) as accp,
            tc.tile_pool(name="fin", bufs=1) as finp,
            tc.tile_pool(name="pmm", bufs=4 if "ttr2" in _KV else 2, space="PSUM") as pmm,
        ):
            nslot = 8
            maxacc = maccp.tile([P, nI * nslot], f32)
            fin = finp.tile([P, nI], f32)
            seye = constp.tile([P, P], f32)
            masks.make_identity(nc, seye[:])
            nc.vector.tensor_scalar_mul(seye[:], seye[:], BIG)

            # ---- gathered blocks: u16-pair xbar straight from DRAM ----
            # bt layout "p (kc r c2)": c2 = 2*col + half is the DoubleRow fp8
            # pair; logical element = feats_n[block row, d], d=2*(kc*128+q)+half
            def prep_b(j, dst):
                dvu = dst.bitcast(u16).rearrange("p (k r c) -> p k r c", k=KC, r=nR)
                gu = gath.bitcast(u16)
                for rt in range(nR):
                    r0 = j * JB + rt * P
                    nc.sync.dma_start_transpose(dvu[:, :, rt, :], gu[r0 : r0 + P, :])

            live = {}

            def prefetch(j, thunks=False):
                bt = btp.tile([P, KC * nR * C2], fp8, name=f"bt{j}", tag="bt")
                live[j] = bt
                dvu = bt.bitcast(u16).rearrange(
                    "p (k r c) -> p k r c", k=KC, r=nR
                )
                gu = gath.bitcast(u16)
                ths = [
                    (lambda rt: lambda: nc.sync.dma_start_transpose(
                        dvu[:, :, rt, :], gu[j * JB + rt * P : j * JB + rt * P + P, :]
                    ))(rt)
                    for rt in range(nR)
                ]
                if thunks:
                    return ths
                for t in ths:
                    t()


            # ---- shard prep: normalize own rows on-chip -> at (block 0) ----
            # at layout "p (kc two r c)": PLANE-separated DoubleRow pairs,
            # element = feats_n[row r*128+c, d = kc*256 + two*128 + q].
            at = atp.tile([P, KC * 2 * nI * P], fp8)
            av = at.rearrange("p (k two r c) -> p k two r c", k=KC, two=2, r=nI)
            ssqb = ssp.tile([P, nR], f32)
            s2b = ssp.tile([P, nR], f32)
            rsb = ssp.tile([P, nR], f32)
            xts = []

            def load_g(g, eng=None):
                x = xp.tile([P, LG * D], fp8, name=f"x{g}", tag="x")
                src = shard[g * LG * P : (g + 1) * LG * P, :]
                (eng or nc.sync).dma_start(
                    out=x.rearrange("p (t d) -> p t d", t=LG),
                    in_=src.rearrange("(t p) d -> p t d", p=P),
                )
                xts.append(x)

            def prep_at_phase1(rt, dma_eng=None):
                g, t = rt // LG, rt % LG
                xsl = xts[g][:, t * D : (t + 1) * D]
                sq = sqp.tile([P, D], fp8, name=f"sq{rt}", tag="sq")
                nc.scalar.activation(
                    sq[:], xsl, AF.Square, accum_out=ssqb[:, rt : rt + 1]
                )
                nc.vector.reciprocal(rsb[:, rt : rt + 1], ssqb[:, rt : rt + 1])
                nc.scalar.activation(
                    s2b[:, rt : rt + 1],
                    rsb[:, rt : rt + 1],
                    AF.Sqrt,
                    scale=SCALE * SCALE,
                )
                xn = xnp.tile([P, D], bf16, name=f"xn{rt}", tag="xn")
                nc.vector.tensor_scalar_mul(xn[:], xsl, s2b[:, rt : rt + 1])
                # bf16 xbar: [128 rows, 1024 d] -> [128 q][k=8][128 rows]
                ts = tstgp.tile([P, D], bf16, name=f"ts{rt}", tag="ts")
                tsv = ts.rearrange("p (k c) -> p k c", k=D // P)
                (dma_eng or nc.sync).dma_start_transpose(tsv[:, :, :], xn[:])
                return tsv

            def prep_at_phase2(rt, tsv):
                # cast-copy into the fp8 plane layout (k = kc*2 + two).
                # NOTE: a gpsimd tensor_copy here faults the device at full
                # scale (NRT_EXEC_UNIT_UNRECOVERABLE) despite passing small
                # tests - keep this on ScalarE.
                nc.scalar.copy(
                    av[:, :, :, rt, :],
                    tsv.rearrange("p (k2 two) c -> p k2 two c", two=2),
                )

            # ---- j loop: blocks 1..7 then 0; within a block, half h0 is
            # staged to bf16 SBUF by ScalarE and half h1 drains on DVE via one
            # tensor_tensor_reduce against the staged half (one PSUM operand
            # per instruction is the ISA limit) ----
            def rhs_maker(j):
                if j == 0:
                    return lambda kc, n: av[:, kc, :, n * 4 : (n + 1) * 4, :]
                sv = live.pop(j).rearrange("p (k r c) -> p k r c", k=KC, r=nR)
                return lambda kc, n: sv[
                    :, kc, n * 4 : (n + 1) * 4, :
                ].rearrange("p r (c two) -> p two (r c)", two=2)

            def mm_round(rhs_of, i, h):
                # half-block round: psum [128, 1024] = 2 chunks (4KB, 2 banks)
                ps = pmm.tile([P, 2 * NCH], f32, name="ps", tag="ps")
                for n2 in range(2):
                    n = h * 2 + n2
                    out = ps[:, n2 * NCH : (n2 + 1) * NCH]
                    for kc in range(KC):
                        nc.tensor.matmul(
                            out,
                            av[:, kc, :, i, :],
                            rhs_of(kc, n),
                            start=(kc == 0),
                            stop=(kc == KC - 1),
                            perf_mode=DR,
                        )
                return ps

            # ---- startup emission: the sync queue is in-order and each DMA
            # holds the single HWDGE ~630ns, so the order here decides when
            # the PE can start. First shard load, then the 8 block-1 xbars
            # the first round needs, then at row-tile 0's chain. ----
            load_g(0)
            bt1thunks = prefetch(1, thunks=True)
            for th in bt1thunks[:4]:
                th()
            load_g(1)
            load_g(2)
            for th in bt1thunks[4:8]:
                th()
            t0 = prep_at_phase1(0)   # at-x0 lands here on the sync queue
            for th in bt1thunks[8:12]:
                th()
            load_g(3)
            for th in bt1thunks[12:]:
                th()
            for g in range(4, nR // LG):
                load_g(g)
            prep_at_phase2(0, t0)
            for rt0 in range(1, 5):
                tt = prep_at_phase1(rt0)
                prep_at_phase2(rt0, tt)

            jorder = list(range(1, nJ)) + [0]
            for jx, j in enumerate(jorder):
                if jx == 0:
                    bt2thunks = prefetch(2, thunks=True)
                elif jx + 1 < nJ and jorder[jx + 1] != 0:
                    prefetch(jorder[jx + 1])
                rhs_of = rhs_maker(j)
                for i in range(nI):
                    if jx == 0:
                        # at row-tile i+5 prepped while PE crunches row-tile i;
                        # bt2 xbars drip-fed so they never head-of-line block
                        # the at xbars on the sync queue
                        if i + 5 < nI:
                            pend = prep_at_phase1(i + 5)
                            pend_rt = i + 5
                        else:
                            pend = None
                        for _ in range(2):
                            if bt2thunks:
                                bt2thunks.pop(0)()
                    if "ttr2" in _KV:
                        # half-pairing drain: h0 staged by ScalarE, h1 drains
                        # via one TTR that folds in the staged half; accum
                        # goes to a dedicated [P,1] tile, then a tiny copy
                        hd = i // 8
                        off = (i * P) % (2 * NCH)
                        psa = mm_round(rhs_of, i, 0)
                        if j == 0 and hd == 0:
                            nc.vector.tensor_add(
                                psa[:, off : off + P],
                                psa[:, off : off + P],
                                seye[:],
                            )
                        st = stgp.tile([P, 2 * NCH], f32, name=f"st{i}", tag="st")
                        nc.scalar.copy(st[:], psa[:])
                        if jx == 0 and pend is not None:
                            prep_at_phase2(pend_rt, pend)
                            pend = None
                        psb = mm_round(rhs_of, i, 1)
                        if j == 0 and hd == 1:
                            nc.vector.tensor_add(
                                psb[:, off : off + P],
                                psb[:, off : off + P],
                                seye[:],
                            )
                        sc = scp.tile([P, 2 * NCH], f32)
                        acct = accp.tile([P, 1], f32, tag="acc")
                        nc.vector.tensor_tensor_reduce(
                            out=sc[:],
                            in0=psb[:],
                            in1=st[:],
                            scale=1.0,
                            scalar=-3.0e38,
                            op0=MAX,
                            op1=MAX,
                            accum_out=acct[:, 0:1],
                        )
                        nc.vector.tensor_copy(
                            maxacc[:, i * 8 + jx : i * 8 + jx + 1], acct[:, 0:1]
                        )
                    else:
                        ps = pmm.tile([P, 4 * NCH], f32, name="ps", tag="ps")
                        for n in range(4):
                            out = ps[:, n * NCH : (n + 1) * NCH]
                            for kc in range(KC):
                                nc.tensor.matmul(
                                    out,
                                    av[:, kc, :, i, :],
                                    rhs_of(kc, n),
                                    start=(kc == 0),
                                    stop=(kc == KC - 1),
                                    perf_mode=DR,
                                )
                        if jx == 0 and pend is not None:
                            prep_at_phase2(pend_rt, pend)
                            pend = None
                        if j == 0:
                            # diag of C: block-0 cols [i*128, (i+1)*128)
                            off = i * P
                            nc.vector.tensor_add(
                                ps[:, off : off + P], ps[:, off : off + P], seye[:]
                            )
                        # drain: ONE DVE reduce per block-row (TTR pairing and
                        # gpsimd copies fault the device at full scale)
                        nc.vector.reduce_max(
                            maxacc[:, i * 8 + jx : i * 8 + jx + 1],
                            ps[:],
                            axis=mybir.AxisListType.X,
                        )
                    if j == 0:
                        # all slots of row-tile i done: finalize right away
                        nc.vector.reduce_max(
                            fin[:, i : i + 1],
                            maxacc[:, i * nslot : (i + 1) * nslot],
                            axis=mybir.AxisListType.X,
                        )
            nc.sync.dma_start(out=outd.rearrange("(i p) -> p i", p=P), in_=fin[:])

    nc.compile()
    return nc


_CACHE = {}


def _get_nc(N, D, NC):
    key = (N, D, NC)
    if key not in _CACHE:
        _CACHE[key] = _build(N, D, NC)
    return _CACHE[key]


def _host_stage(feats, NC):
    """Quantize to fp8 and build per-core inputs: raw fp8 shard + the
    all-gather result (normalized, scaled, fp8) rotated shard-first."""
    import ml_dtypes

    f8 = ml_dtypes.float8_e4m3
    N = feats.shape[0]
    SH = N // NC
    q = feats.astype(f8)
    qf = q.astype(np.float32)
    s = SCALE / np.sqrt((qf * qf).sum(axis=1, keepdims=True))
    xn = (qf * s).astype(f8)  # the all-gathered normalized features
    maps = []
    for c in range(NC):
        maps.append(
            {
                "gathered": np.ascontiguousarray(np.roll(xn, -c * SH, axis=0)),
                "shard": np.ascontiguousarray(q[c * SH : (c + 1) * SH]),
            }
        )
    return maps


def _loss_from_maxdot(m):
    maxcos = m.astype(np.float64) / (SCALE * SCALE)
    dist = np.sqrt(np.maximum(2.0 - 2.0 * maxcos, 0.0))
    return np.asarray(-np.mean(np.log(dist + 1e-8)), dtype=np.float32)


def kernel(features):
    from concourse.bass_utils import run_bass_kernel_spmd

    feats = np.asarray(features, dtype=np.float32)
    N, D = feats.shape
    nc = _get_nc(N, D, NCORES)
    res = run_bass_kernel_spmd(nc, _host_stage(feats, NCORES), list(range(NCORES)))
    m = np.concatenate([res.results[c]["maxdot"] for c in range(NCORES)])
    return _loss_from_maxdot(m)


# revision 51
# speedup vs baseline: 1.0273x; 1.0273x over previous
"""KoLeo loss kernel for Trainium2 (8 NeuronCores, SPMD row-sharded), fp8 edition.

Algorithm (matches the jax reference):
  feats_n = features / ||features||_row          (L2 row normalize)
  C       = feats_n @ feats_n.T                  (cosine similarity, NxN)
  m_i     = max_{j != i} C[i, j]                 (nearest-neighbor cosine)
  dist_i  = sqrt(2 - 2 m_i)                      (= ||f_i - f_j*|| for unit vectors)
  loss    = -mean(log(dists_i + 1e-8))

Distribution follows the torch/dist decomposition from the sharding hint:
each device L2-normalizes its OWN 2048-row shard on-chip, all-gathers the
normalized features (host plays the interconnect for the gather, staging the
quantized fp8 gather result in each core's DRAM), computes its 2048 x 16384
slice of the similarity against the gathered set, reduces a per-row max, and
the tiny final sqrt/log/mean runs on host.

Device pipeline per core (SPMD, identical program):
  - Inputs are rotated so the core's own shard is block 0; the C diagonal
    then sits in block 0 at fixed positions (16 constant masked adds).
  - Shard prep per row-tile: ScalarE Square+accum -> DVE reciprocal ->
    ScalarE Sqrt(scale) -> ScalarE normalize (fp8 -> bf16), bf16 DMA-xbar
    transpose, ScalarE cast-copy into the fp8 PLANE-separated layout that
    LdWeights dual-fp8 mode requires for the stationary operand.
  - Gathered blocks 1..7 ride the DMA xbar straight out of DRAM with
    consecutive fp8 pairs packed as uint16 - the pair-interleaved layout is
    accepted for the MOVING operand of DoubleRow matmuls.
  - Matmuls run fp8 DoubleRow (2 PE rows/cycle, K=256 per instruction):
    4x fewer PE cycles than bf16.
  - Row-max drain alternates blocks: odd blocks are staged PSUM->bf16 SBUF
    by ScalarE; even blocks drain on DVE via ONE tensor_tensor_reduce that
    maxes the live PSUM block against the staged block and reduces - each
    DVE pass retires two blocks (the ISA allows only one PSUM operand per
    instruction, so this is the cheapest legal drain).
"""

import os

import numpy as np

_KV = os.environ.get("KV", "")  # debug feature flags, comma separated

P = 128          # SBUF partitions
N_FULL = 16384
D_FULL = 1024
NCORES = 8
SCALE = 16.0     # fp8 range scale for normalized rows; dots come out x SCALE^2
BIG = -1024.0    # diagonal mask add (dots are ~ +-256 after scaling)


def _build(N, D, NC):
    import concourse.bacc as bacc
    import concourse.mybir as mybir
    from concourse import tile

    from concourse import masks

    f32 = mybir.dt.float32
    bf16 = mybir.dt.bfloat16
    fp8 = mybir.dt.float8e4
    u16 = mybir.dt.uint16
    AF = mybir.ActivationFunctionType
    DR = mybir.MatmulPerfMode.DoubleRow
    MAX = mybir.AluOpType.max

    SH = N // NC          # shard rows per core (2048)
    JB = SH               # column-block size
    nJ = N // JB          # column blocks (8)
    nR = JB // P          # row tiles per block (16)
    nI = nR               # row tiles in shard (16)
    KC = D // 256         # 256-deep DoubleRow super-chunks (4)
    C2 = 256              # fp8 elems per (kc, row-tile) = 2 x 128
    NCH = 512             # psum chunk cols
    LG = 2                # row-tiles per shard DMA load

    nc = bacc.Bacc("TRN2", target_bir_lowering=False, debug=False)
    # all-gathered normalized features (fp8, x SCALE), rotated shard-first
    gath = nc.dram_tensor("gathered", [N, D], fp8, kind="ExternalInput").ap()
    # raw (unnormalized) fp8 shard rows
    shard = nc.dram_tensor("shard", [SH, D], fp8, kind="ExternalInput").ap()
    outd = nc.dram_tensor("maxdot", [SH], f32, kind="ExternalOutput").ap()

    with tile.TileContext(nc) as tc:
        with (
            tc.tile_pool(name="const", bufs=1) as constp,
            tc.tile_pool(name="at", bufs=1) as atp,
            tc.tile_pool(name="bt", bufs=3) as btp,
            tc.tile_pool(name="x", bufs=8) as xp,
            tc.tile_pool(name="sq", bufs=4) as sqp,
            tc.tile_pool(name="ss", bufs=1) as ssp,
            tc.tile_pool(name="xn", bufs=6) as xnp,
            tc.tile_pool(name="tstg", bufs=6) as tstgp,
            tc.tile_pool(name="stg", bufs=3) as stgp,
            tc.tile_pool(name="sc", bufs=3) as scp,
            tc.tile_pool(name="macc", bufs=1) as maccp,
            tc.tile_pool(name="acc", bufs=4) as accp,
            tc.tile_pool(name="fin", bufs=1) as finp,
            tc.tile_pool(name="pmm", bufs=4 if "ttr2" in _KV else 2, space="PSUM") as pmm,
        ):
            nslot = 8
            maxacc = maccp.tile([P, nI * nslot], f32)
            fin = finp.tile([P, nI], f32)
            seye = constp.tile([P, P], f32)
            masks.make_identity(nc, seye[:])
            nc.vector.tensor_scalar_mul(seye[:], seye[:], BIG)
            identb = constp.tile([P, P], bf16)
            masks.make_identity(nc, identb[:])

            # ---- gathered blocks: u16-pair xbar straight from DRAM ----
            # bt layout "p (kc r c2)": c2 = 2*col + half is the DoubleRow fp8
            # pair; logical element = feats_n[block row, d], d=2*(kc*128+q)+half
            def prep_b(j, dst):
                dvu = dst.bitcast(u16).rearrange("p (k r c) -> p k r c", k=KC, r=nR)
                gu = gath.bitcast(u16)
                for rt in range(nR):
                    r0 = j * JB + rt * P
                    nc.sync.dma_start_transpose(dvu[:, :, rt, :], gu[r0 : r0 + P, :])

            live = {}

            def prefetch(j, thunks=False):
                bt = btp.tile([P, KC * nR * C2], fp8, name=f"bt{j}", tag="bt")
                live[j] = bt
                dvu = bt.bitcast(u16).rearrange(
                    "p (k r c) -> p k r c", k=KC, r=nR
                )
                gu = gath.bitcast(u16)
                ths = [
                    (lambda rt: lambda: nc.sync.dma_start_transpose(
                        dvu[:, :, rt, :], gu[j * JB + rt * P : j * JB + rt * P + P, :]
                    ))(rt)
                    for rt in range(nR)
                ]
                if thunks:
                    return ths
                for t in ths:
                    t()


            # ---- shard prep: normalize own rows on-chip -> at (block 0) ----
            # at layout "p (kc two r c)": PLANE-separated DoubleRow pairs,
            # element = feats_n[row r*128+c, d = kc*256 + two*128 + q].
            at = atp.tile([P, KC * 2 * nI * P], fp8)
            av = at.rearrange("p (k two r c) -> p k two r c", k=KC, two=2, r=nI)
            ssqb = ssp.tile([P, nR], f32)
            s2b = ssp.tile([P, nR], f32)
            rsb = ssp.tile([P, nR], f32)
            xts = []

            def load_g(g, eng=None):
                x = xp.tile([P, LG * D], fp8, name=f"x{g}", tag="x")
                src = shard[g * LG * P : (g + 1) * LG * P, :]
                (eng or nc.sync).dma_start(
                    out=x.rearrange("p (t d) -> p t d", t=LG),
                    in_=src.rearrange("(t p) d -> p t d", p=P),
                )
                xts.append(x)

            def prep_at_phase1(rt, dma_eng=None):
                g, t = rt // LG, rt % LG
                xsl = xts[g][:, t * D : (t + 1) * D]
                sq = sqp.tile([P, D], fp8, name=f"sq{rt}", tag="sq")
                nc.scalar.activation(
                    sq[:], xsl, AF.Square, accum_out=ssqb[:, rt : rt + 1]
                )
                nc.vector.reciprocal(rsb[:, rt : rt + 1], ssqb[:, rt : rt + 1])
                nc.scalar.activation(
                    s2b[:, rt : rt + 1],
                    rsb[:, rt : rt + 1],
                    AF.Sqrt,
                    scale=SCALE * SCALE,
                )
                xn = xnp.tile([P, D], bf16, name=f"xn{rt}", tag="xn")
                nc.vector.tensor_scalar_mul(xn[:], xsl, s2b[:, rt : rt + 1])
                # bf16 xbar: [128 rows, 1024 d] -> [128 q][k=8][128 rows]
                ts = tstgp.tile([P, D], bf16, name=f"ts{rt}", tag="ts")
                tsv = ts.rearrange("p (k c) -> p k c", k=D // P)
                (dma_eng or nc.sync).dma_start_transpose(tsv[:, :, :], xn[:])
                return tsv

            def prep_at_pe(rt):
                # startup-only variant: transpose on the (idle) TensorEngine
                # instead of the congested sync DMA queue; also warms the PE
                # clock p-state before the matmul stream begins
                g, t = rt // LG, rt % LG
                xsl = xts[g][:, t * D : (t + 1) * D]
                sq = sqp.tile([P, D], fp8, name=f"sq{rt}", tag="sq")
                nc.scalar.activation(
                    sq[:], xsl, AF.Square, accum_out=ssqb[:, rt : rt + 1]
                )
                nc.vector.reciprocal(rsb[:, rt : rt + 1], ssqb[:, rt : rt + 1])
                nc.scalar.activation(
                    s2b[:, rt : rt + 1],
                    rsb[:, rt : rt + 1],
                    AF.Sqrt,
                    scale=SCALE * SCALE,
                )
                xn = xnp.tile([P, D], bf16, name=f"xn{rt}", tag="xn")
                nc.vector.tensor_scalar_mul(xn[:], xsl, s2b[:, rt : rt + 1])
                tp = pmm.tile([P, D], bf16, name=f"tp{rt}", tag="ps")
                tpv = tp.rearrange("p (k c) -> p k c", k=D // P)
                for k in range(D // P):
                    nc.tensor.transpose(
                        tpv[:, k, :], xn[:, k * P : (k + 1) * P], identb[:]
                    )
                nc.scalar.copy(
                    av[:, :, :, rt, :],
                    tpv.rearrange("p (k2 two) c -> p k2 two c", two=2),
                )

            def prep_at_phase2(rt, tsv):
                # cast-copy into the fp8 plane layout (k = kc*2 + two).
                # NOTE: a gpsimd tensor_copy here faults the device at full
                # scale (NRT_EXEC_UNIT_UNRECOVERABLE) despite passing small
                # tests - keep this on ScalarE.
                nc.scalar.copy(
                    av[:, :, :, rt, :],
                    tsv.rearrange("p (k2 two) c -> p k2 two c", two=2),
                )

            # ---- j loop: blocks 1..7 then 0; within a block, half h0 is
            # staged to bf16 SBUF by ScalarE and half h1 drains on DVE via one
            # tensor_tensor_reduce against the staged half (one PSUM operand
            # per instruction is the ISA limit) ----
            def rhs_maker(j):
                if j == 0:
                    return lambda kc, n: av[:, kc, :, n * 4 : (n + 1) * 4, :]
                sv = live.pop(j).rearrange("p (k r c) -> p k r c", k=KC, r=nR)
                return lambda kc, n: sv[
                    :, kc, n * 4 : (n + 1) * 4, :
                ].rearrange("p r (c two) -> p two (r c)", two=2)

            def mm_round(rhs_of, i, h):
                # half-block round: psum [128, 1024] = 2 chunks (4KB, 2 banks)
                ps = pmm.tile([P, 2 * NCH], f32, name="ps", tag="ps")
                for n2 in range(2):
                    n = h * 2 + n2
                    out = ps[:, n2 * NCH : (n2 + 1) * NCH]
                    for kc in range(KC):
                        nc.tensor.matmul(
                            out,
                            av[:, kc, :, i, :],
                            rhs_of(kc, n),
                            start=(kc == 0),
                            stop=(kc == KC - 1),
                            perf_mode=DR,
                        )
                return ps

            # ---- startup emission: the sync queue is in-order and each DMA
            # holds the single HWDGE ~630ns, so the order here decides when
            # the PE can start. First shard load, then the 8 block-1 xbars
            # the first round needs, then at row-tile 0's chain. ----
            load_g(0)
            bt1thunks = prefetch(1, thunks=True)
            for th in bt1thunks[:8]:
                th()
            load_g(1)
            load_g(2)
            for th in bt1thunks[8:]:
                th()
            load_g(3)
            for g in range(4, nR // LG):
                load_g(g)
            # rt0-2 via the TensorEngine (idle during startup, warms the
            # p-state ramp); rt3-4 via the now-quieter sync DMA queue
            for rt0 in range(3):
                prep_at_pe(rt0)
            for rt0 in range(3, 5):
                tt = prep_at_phase1(rt0)
                prep_at_phase2(rt0, tt)

            jorder = list(range(1, nJ)) + [0]
            for jx, j in enumerate(jorder):
                if jx == 0:
                    bt2thunks = prefetch(2, thunks=True)
                elif jx + 1 < nJ and jorder[jx + 1] != 0:
                    prefetch(jorder[jx + 1])
                rhs_of = rhs_maker(j)
                for i in range(nI):
                    if jx == 0:
                        # at row-tile i+5 prepped while PE crunches row-tile i;
                        # bt2 xbars drip-fed so they never head-of-line block
                        # the at xbars on the sync queue
                        if i + 5 < nI:
                            pend = prep_at_phase1(i + 5)
                            pend_rt = i + 5
                        else:
                            pend = None
                        for _ in range(2):
                            if bt2thunks:
                                bt2thunks.pop(0)()
                    if "ttr2" in _KV:
                        # half-pairing drain: h0 staged by ScalarE, h1 drains
                        # via one TTR that folds in the staged half; accum
                        # goes to a dedicated [P,1] tile, then a tiny copy
                        hd = i // 8
                        off = (i * P) % (2 * NCH)
                        psa = mm_round(rhs_of, i, 0)
                        if j == 0 and hd == 0:
                            nc.vector.tensor_add(
                                psa[:, off : off + P],
                                psa[:, off : off + P],
                                seye[:],
                            )
                        st = stgp.tile([P, 2 * NCH], f32, name=f"st{i}", tag="st")
                        nc.scalar.copy(st[:], psa[:])
                        if jx == 0 and pend is not None:
                            prep_at_phase2(pend_rt, pend)
                            pend = None
                        psb = mm_round(rhs_of, i, 1)
                        if j == 0 and hd == 1:
                            nc.vector.tensor_add(
                                psb[:, off : off + P],
                                psb[:, off : off + P],
                                seye[:],
                            )
                        sc = scp.tile([P, 2 * NCH], f32)
                        acct = accp.tile([P, 1], f32, tag="acc")
                        nc.vector.tensor_tensor_reduce(
                            out=sc[:],
                            in0=psb[:],
                            in1=st[:],
                            scale=1.0,
                            scalar=-3.0e38,
                            op0=MAX,
                            op1=MAX,
                            accum_out=acct[:, 0:1],
                        )
                        nc.vector.tensor_copy(
                            maxacc[:, i * 8 + jx : i * 8 + jx + 1], acct[:, 0:1]
                        )
                    else:
                        ps = pmm.tile([P, 4 * NCH], f32, name="ps", tag="ps")
                        for n in range(4):
                            out = ps[:, n * NCH : (n + 1) * NCH]
                            for kc in range(KC):
                                nc.tensor.matmul(
                                    out,
                                    av[:, kc, :, i, :],
                                    rhs_of(kc, n),
                                    start=(kc == 0),
                                    stop=(kc == KC - 1),
                                    perf_mode=DR,
                                )
                        if jx == 0 and pend is not None:
                            prep_at_phase2(pend_rt, pend)
                            pend = None
                        if j == 0:
                            # diag of C: block-0 cols [i*128, (i+1)*128)
                            off = i * P
                            nc.vector.tensor_add(
                                ps[:, off : off + P], ps[:, off : off + P], seye[:]
                            )
                        # drain: ONE DVE reduce per block-row (TTR pairing and
                        # gpsimd copies fault the device at full scale)
                        nc.vector.reduce_max(
                            maxacc[:, i * 8 + jx : i * 8 + jx + 1],
                            ps[:],
                            axis=mybir.AxisListType.X,
                        )
                    if j == 0:
                        # all slots of row-tile i done: finalize right away
                        nc.vector.reduce_max(
                            fin[:, i : i + 1],
                            maxacc[:, i * nslot : (i + 1) * nslot],
                            axis=mybir.AxisListType.X,
                        )
            nc.sync.dma_start(out=outd.rearrange("(i p) -> p i", p=P), in_=fin[:])

    nc.compile()
    return nc


_CACHE = {}


def _get_nc(N, D, NC):
    key = (N, D, NC)
    if key not in _CACHE:
        _CACHE[key] = _build(N, D, NC)
    return _CACHE[key]


def _host_stage(feats, NC):
    """Quantize to fp8 and build per-core inputs: raw fp8 shard + the
    all-gather result (normalized, scaled, fp8) rotated shard-first."""
    import ml_dtypes

    f8 = ml_dtypes.float8_e4m3
    N = feats.shape[0]
    SH = N // NC
    q = feats.astype(f8)
    qf = q.astype(np.float32)
    s = SCALE / np.sqrt((qf * qf).sum(axis=1, keepdims=True))
    xn = (qf * s).astype(f8)  # the all-gathered normalized features
    maps = []
    for c in range(NC):
        maps.append(
            {
                "gathered": np.ascontiguousarray(np.roll(xn, -c * SH, axis=0)),
                "shard": np.ascontiguousarray(q[c * SH : (c + 1) * SH]),
            }
        )
    return maps


def _loss_from_maxdot(m):
    maxcos = m.astype(np.float64) / (SCALE * SCALE)
    dist = np.sqrt(np.maximum(2.0 - 2.0 * maxcos, 0.0))
    return np.asarray(-np.mean(np.log(dist + 1e-8)), dtype=np.float32)


def kernel(features):
    from concourse.bass_utils import run_bass_kernel_spmd

    feats = np.asarray(features, dtype=np.float32)
    N, D = feats.shape
    nc = _get_nc(N, D, NCORES)
    res = run_bass_kernel_spmd(nc, _host_stage(feats, NCORES), list(range(NCORES)))
    m = np.concatenate([res.results[c]["maxdot"] for c in range(NCORES)])
    return _loss_from_maxdot(m)


# revision 56
# speedup vs baseline: 1.1230x; 1.0932x over previous
"""KoLeo loss kernel for Trainium2 (8 NeuronCores, SPMD row-sharded), fp8 edition.

Algorithm (matches the jax reference):
  feats_n = features / ||features||_row          (L2 row normalize)
  C       = feats_n @ feats_n.T                  (cosine similarity, NxN)
  m_i     = max_{j != i} C[i, j]                 (nearest-neighbor cosine)
  dist_i  = sqrt(2 - 2 m_i)                      (= ||f_i - f_j*|| for unit vectors)
  loss    = -mean(log(dists_i + 1e-8))

Distribution follows the torch/dist decomposition from the sharding hint:
each device L2-normalizes its OWN 2048-row shard on-chip, all-gathers the
normalized features (host plays the interconnect for the gather, staging the
quantized fp8 gather result in each core's DRAM), computes its 2048 x 16384
slice of the similarity against the gathered set, reduces a per-row max, and
the tiny final sqrt/log/mean runs on host.

Device pipeline per core (SPMD, identical program):
  - Inputs are rotated so the core's own shard is block 0; the C diagonal
    then sits in block 0 at fixed positions (16 constant masked adds).
  - Shard prep per row-tile: ScalarE Square+accum -> DVE reciprocal ->
    ScalarE Sqrt(scale) -> ScalarE normalize (fp8 -> bf16), bf16 DMA-xbar
    transpose, ScalarE cast-copy into the fp8 PLANE-separated layout that
    LdWeights dual-fp8 mode requires for the stationary operand.
  - Gathered blocks 1..7 ride the DMA xbar straight out of DRAM with
    consecutive fp8 pairs packed as uint16 - the pair-interleaved layout is
    accepted for the MOVING operand of DoubleRow matmuls.
  - Matmuls run fp8 DoubleRow (2 PE rows/cycle, K=256 per instruction):
    4x fewer PE cycles than bf16.
  - Row-max drain alternates blocks: odd blocks are staged PSUM->bf16 SBUF
    by ScalarE; even blocks drain on DVE via ONE tensor_tensor_reduce that
    maxes the live PSUM block against the staged block and reduces - each
    DVE pass retires two blocks (the ISA allows only one PSUM operand per
    instruction, so this is the cheapest legal drain).
"""

import os

import numpy as np

_KV = os.environ.get("KV", "")  # debug feature flags, comma separated

P = 128          # SBUF partitions
N_FULL = 16384
D_FULL = 1024
NCORES = 8
SCALE = 16.0     # fp8 range scale for normalized rows; dots come out x SCALE^2
BIG = -1024.0    # diagonal mask add (dots are ~ +-256 after scaling)


def _build(N, D, NC):
    import concourse.bacc as bacc
    import concourse.mybir as mybir
    from concourse import tile

    from concourse import masks

    f32 = mybir.dt.float32
    bf16 = mybir.dt.bfloat16
    fp8 = mybir.dt.float8e4
    u16 = mybir.dt.uint16
    AF = mybir.ActivationFunctionType
    DR = mybir.MatmulPerfMode.DoubleRow
    MAX = mybir.AluOpType.max

    SH = N // NC          # shard rows per core (2048)
    JB = SH               # column-block size
    nJ = N // JB          # column blocks (8)
    nR = JB // P          # row tiles per block (16)
    nI = nR               # row tiles in shard (16)
    KC = D // 256         # 256-deep DoubleRow super-chunks (4)
    C2 = 256              # fp8 elems per (kc, row-tile) = 2 x 128
    NCH = 512             # psum chunk cols
    LG = 2                # row-tiles per shard DMA load

    nc = bacc.Bacc("TRN2", target_bir_lowering=False, debug=False)
    # all-gathered normalized features (fp8, x SCALE), rotated shard-first
    gath = nc.dram_tensor("gathered", [N, D], fp8, kind="ExternalInput").ap()
    # raw (unnormalized) fp8 shard rows
    shard = nc.dram_tensor("shard", [SH, D], fp8, kind="ExternalInput").ap()
    outd = nc.dram_tensor("maxdot", [SH], f32, kind="ExternalOutput").ap()

    with tile.TileContext(nc) as tc:
        with (
            tc.tile_pool(name="const", bufs=1) as constp,
            tc.tile_pool(name="at", bufs=1) as atp,
            tc.tile_pool(name="bt", bufs=3) as btp,
            tc.tile_pool(name="x", bufs=8) as xp,
            tc.tile_pool(name="sq", bufs=4) as sqp,
            tc.tile_pool(name="ss", bufs=1) as ssp,
            tc.tile_pool(name="xn", bufs=6) as xnp,
            tc.tile_pool(name="tstg", bufs=6) as tstgp,
            tc.tile_pool(name="stg", bufs=3) as stgp,
            tc.tile_pool(name="sc", bufs=3) as scp,
            tc.tile_pool(name="macc", bufs=1) as maccp,
            tc.tile_pool(name="acc", bufs=4) as accp,
            tc.tile_pool(name="fin", bufs=1) as finp,
            tc.tile_pool(name="pmm", bufs=4 if ("ttr2" in _KV or "ttc" in _KV) else 2, space="PSUM") as pmm,
        ):
            nslot = 8
            maxacc = maccp.tile([P, nI * nslot], f32)
            fin = finp.tile([P, nI], f32)
            HW_ = 2 * NCH  # half-block width (1024)
            car = maccp.tile([P, nI * HW_], bf16)  # per-i elementwise carries
            seye = constp.tile([P, P], f32)
            masks.make_identity(nc, seye[:])
            nc.vector.tensor_scalar_mul(seye[:], seye[:], BIG)
            identb = constp.tile([P, P], bf16)
            masks.make_identity(nc, identb[:])

            # ---- gathered blocks: u16-pair xbar straight from DRAM ----
            # bt layout "p (kc r c2)": c2 = 2*col + half is the DoubleRow fp8
            # pair; logical element = feats_n[block row, d], d=2*(kc*128+q)+half
            def prep_b(j, dst):
                dvu = dst.bitcast(u16).rearrange("p (k r c) -> p k r c", k=KC, r=nR)
                gu = gath.bitcast(u16)
                for rt in range(nR):
                    r0 = j * JB + rt * P
                    nc.sync.dma_start_transpose(dvu[:, :, rt, :], gu[r0 : r0 + P, :])

            live = {}

            def prefetch(j, thunks=False):
                bt = btp.tile([P, KC * nR * C2], fp8, name=f"bt{j}", tag="bt")
                live[j] = bt
                dvu = bt.bitcast(u16).rearrange(
                    "p (k r c) -> p k r c", k=KC, r=nR
                )
                gu = gath.bitcast(u16)
                ths = [
                    (lambda rt: lambda: nc.sync.dma_start_transpose(
                        dvu[:, :, rt, :], gu[j * JB + rt * P : j * JB + rt * P + P, :]
                    ))(rt)
                    for rt in range(nR)
                ]
                if thunks:
                    return ths
                for t in ths:
                    t()


            # ---- shard prep: normalize own rows on-chip -> at (block 0) ----
            # at layout "p (kc two r c)": PLANE-separated DoubleRow pairs,
            # element = feats_n[row r*128+c, d = kc*256 + two*128 + q].
            at = atp.tile([P, KC * 2 * nI * P], fp8)
            av = at.rearrange("p (k two r c) -> p k two r c", k=KC, two=2, r=nI)
            ssqb = ssp.tile([P, nR], f32)
            s2b = ssp.tile([P, nR], f32)
            rsb = ssp.tile([P, nR], f32)
            xts = []

            def load_g(g, eng=None):
                x = xp.tile([P, LG * D], fp8, name=f"x{g}", tag="x")
                src = shard[g * LG * P : (g + 1) * LG * P, :]
                (eng or nc.sync).dma_start(
                    out=x.rearrange("p (t d) -> p t d", t=LG),
                    in_=src.rearrange("(t p) d -> p t d", p=P),
                )
                xts.append(x)

            def prep_at_phase1(rt, dma_eng=None):
                g, t = rt // LG, rt % LG
                xsl = xts[g][:, t * D : (t + 1) * D]
                sq = sqp.tile([P, D], fp8, name=f"sq{rt}", tag="sq")
                nc.scalar.activation(
                    sq[:], xsl, AF.Square, accum_out=ssqb[:, rt : rt + 1]
                )
                nc.vector.reciprocal(rsb[:, rt : rt + 1], ssqb[:, rt : rt + 1])
                nc.scalar.activation(
                    s2b[:, rt : rt + 1],
                    rsb[:, rt : rt + 1],
                    AF.Sqrt,
                    scale=SCALE * SCALE,
                )
                xn = xnp.tile([P, D], bf16, name=f"xn{rt}", tag="xn")
                nc.vector.tensor_scalar_mul(xn[:], xsl, s2b[:, rt : rt + 1])
                # bf16 xbar: [128 rows, 1024 d] -> [128 q][k=8][128 rows]
                ts = tstgp.tile([P, D], bf16, name=f"ts{rt}", tag="ts")
                tsv = ts.rearrange("p (k c) -> p k c", k=D // P)
                (dma_eng or nc.sync).dma_start_transpose(tsv[:, :, :], xn[:])
                return tsv

            def prep_at_pe(rt):
                # startup-only variant: transpose on the (idle) TensorEngine
                # instead of the congested sync DMA queue; also warms the PE
                # clock p-state before the matmul stream begins
                g, t = rt // LG, rt % LG
                xsl = xts[g][:, t * D : (t + 1) * D]
                sq = sqp.tile([P, D], fp8, name=f"sq{rt}", tag="sq")
                nc.scalar.activation(
                    sq[:], xsl, AF.Square, accum_out=ssqb[:, rt : rt + 1]
                )
                nc.vector.reciprocal(rsb[:, rt : rt + 1], ssqb[:, rt : rt + 1])
                nc.scalar.activation(
                    s2b[:, rt : rt + 1],
                    rsb[:, rt : rt + 1],
                    AF.Sqrt,
                    scale=SCALE * SCALE,
                )
                xn = xnp.tile([P, D], bf16, name=f"xn{rt}", tag="xn")
                nc.vector.tensor_scalar_mul(xn[:], xsl, s2b[:, rt : rt + 1])
                tp = pmm.tile([P, D], bf16, name=f"tp{rt}", tag="ps")
                tpv = tp.rearrange("p (k c) -> p k c", k=D // P)
                for k in range(D // P):
                    nc.tensor.transpose(
                        tpv[:, k, :], xn[:, k * P : (k + 1) * P], identb[:]
                    )
                nc.scalar.copy(
                    av[:, :, :, rt, :],
                    tpv.rearrange("p (k2 two) c -> p k2 two c", two=2),
                )

            def prep_at_phase2(rt, tsv):
                # cast-copy into the fp8 plane layout (k = kc*2 + two).
                # NOTE: a gpsimd tensor_copy here faults the device at full
                # scale (NRT_EXEC_UNIT_UNRECOVERABLE) despite passing small
                # tests - keep this on ScalarE.
                nc.scalar.copy(
                    av[:, :, :, rt, :],
                    tsv.rearrange("p (k2 two) c -> p k2 two c", two=2),
                )

            # ---- j loop: blocks 1..7 then 0; within a block, half h0 is
            # staged to bf16 SBUF by ScalarE and half h1 drains on DVE via one
            # tensor_tensor_reduce against the staged half (one PSUM operand
            # per instruction is the ISA limit) ----
            def rhs_maker(j):
                if j == 0:
                    return lambda kc, n: av[:, kc, :, n * 4 : (n + 1) * 4, :]
                sv = live.pop(j).rearrange("p (k r c) -> p k r c", k=KC, r=nR)
                return lambda kc, n: sv[
                    :, kc, n * 4 : (n + 1) * 4, :
                ].rearrange("p r (c two) -> p two (r c)", two=2)

            def mm_round(rhs_of, i, h):
                # half-block round: psum [128, 1024] = 2 chunks (4KB, 2 banks)
                ps = pmm.tile([P, 2 * NCH], f32, name="ps", tag="ps")
                for n2 in range(2):
                    n = h * 2 + n2
                    out = ps[:, n2 * NCH : (n2 + 1) * NCH]
                    for kc in range(KC):
                        nc.tensor.matmul(
                            out,
                            av[:, kc, :, i, :],
                            rhs_of(kc, n),
                            start=(kc == 0),
                            stop=(kc == KC - 1),
                            perf_mode=DR,
                        )
                return ps

            # ---- startup emission: the sync queue is in-order and each DMA
            # holds the single HWDGE ~630ns, so the order here decides when
            # the PE can start. First shard load, then the 8 block-1 xbars
            # the first round needs, then at row-tile 0's chain. ----
            load_g(0)
            bt1thunks = prefetch(1, thunks=True)
            for th in bt1thunks[:8]:
                th()
            load_g(1)
            load_g(2)
            for th in bt1thunks[8:]:
                th()
            load_g(3)
            for g in range(4, nR // LG):
                load_g(g)
            # rt0-2 via the TensorEngine (idle during startup, warms the
            # p-state ramp); rt3-4 via the now-quieter sync DMA queue
            for rt0 in range(3):
                prep_at_pe(rt0)
            for rt0 in range(3, 5):
                tt = prep_at_phase1(rt0)
                prep_at_phase2(rt0, tt)

            jorder = list(range(1, nJ)) + [0]
            for jx, j in enumerate(jorder):
                if jx == 0:
                    bt2thunks = prefetch(2, thunks=True)
                elif jx + 1 < nJ and jorder[jx + 1] != 0:
                    prefetch(jorder[jx + 1])
                rhs_of = rhs_maker(j)
                for i in range(nI):
                    if jx == 0:
                        # at row-tile i+5 prepped while PE crunches row-tile i;
                        # bt2 xbars drip-fed so they never head-of-line block
                        # the at xbars on the sync queue
                        if i + 5 < nI:
                            pend = prep_at_phase1(i + 5)
                            pend_rt = i + 5
                        else:
                            pend = None
                        for _ in range(2):
                            if bt2thunks:
                                bt2thunks.pop(0)()
                    if "ttc" in _KV:
                        # TT-carry drain: stage h0 to bf16 SBUF (ScalarE),
                        # pair-max h1-PSUM against it with plain tensor_max
                        # (the opcode the diag adds prove safe), fold into a
                        # bf16 carry at the DVE 2x SBUF rate; one reduce per
                        # row-tile at the end.
                        hd = i // 8
                        off = (i * P) % HW_
                        psa = mm_round(rhs_of, i, 0)
                        if j == 0 and hd == 0:
                            nc.vector.tensor_add(
                                psa[:, off : off + P],
                                psa[:, off : off + P],
                                seye[:],
                            )
                        st = stgp.tile([P, HW_], bf16, name=f"stc{i}", tag="st")
                        nc.scalar.copy(st[:], psa[:])
                        if jx == 0 and pend is not None:
                            prep_at_phase2(pend_rt, pend)
                            pend = None
                        psb = mm_round(rhs_of, i, 1)
                        if j == 0 and hd == 1:
                            nc.vector.tensor_add(
                                psb[:, off : off + P],
                                psb[:, off : off + P],
                                seye[:],
                            )
                        carsl = car[:, i * HW_ : (i + 1) * HW_]
                        if jx == 0:
                            nc.vector.tensor_max(carsl, psb[:], st[:])
                        else:
                            cmb = scp.tile([P, HW_], bf16, tag="cmb")
                            nc.vector.tensor_max(cmb[:], psb[:], st[:])
                            nc.vector.tensor_max(carsl, carsl, cmb[:])
                        if j == 0:
                            nc.vector.reduce_max(
                                fin[:, i : i + 1],
                                carsl,
                                axis=mybir.AxisListType.X,
                            )
                    elif "ttr2" in _KV:
                        # half-pairing drain: h0 staged by ScalarE, h1 drains
                        # via one TTR that folds in the staged half; accum
                        # goes to a dedicated [P,1] tile, then a tiny copy
                        hd = i // 8
                        off = (i * P) % (2 * NCH)
                        psa = mm_round(rhs_of, i, 0)
                        if j == 0 and hd == 0:
                            nc.vector.tensor_add(
                                psa[:, off : off + P],
                                psa[:, off : off + P],
                                seye[:],
                            )
                        st = stgp.tile([P, 2 * NCH], f32, name=f"st{i}", tag="st")
                        nc.scalar.copy(st[:], psa[:])
                        if jx == 0 and pend is not None:
                            prep_at_phase2(pend_rt, pend)
                            pend = None
                        psb = mm_round(rhs_of, i, 1)
                        if j == 0 and hd == 1:
                            nc.vector.tensor_add(
                                psb[:, off : off + P],
                                psb[:, off : off + P],
                                seye[:],
                            )
                        sc = scp.tile([P, 2 * NCH], f32)
                        acct = accp.tile([P, 1], f32, tag="acc")
                        nc.vector.tensor_tensor_reduce(
                            out=sc[:],
                            in0=psb[:],
                            in1=st[:],
                            scale=1.0,
                            scalar=-3.0e38,
                            op0=MAX,
                            op1=MAX,
                            accum_out=acct[:, 0:1],
                        )
                        nc.vector.tensor_copy(
                            maxacc[:, i * 8 + jx : i * 8 + jx + 1], acct[:, 0:1]
                        )
                    else:
                        ps = pmm.tile([P, 4 * NCH], f32, name="ps", tag="ps")
                        for n in range(4):
                            out = ps[:, n * NCH : (n + 1) * NCH]
                            for kc in range(KC):
                                nc.tensor.matmul(
                                    out,
                                    av[:, kc, :, i, :],
                                    rhs_of(kc, n),
                                    start=(kc == 0),
                                    stop=(kc == KC - 1),
                                    perf_mode=DR,
                                )
                        if jx == 0 and pend is not None:
                            prep_at_phase2(pend_rt, pend)
                            pend = None
                        if j == 0:
                            # diag of C: block-0 cols [i*128, (i+1)*128)
                            off = i * P
                            nc.vector.tensor_add(
                                ps[:, off : off + P], ps[:, off : off + P], seye[:]
                            )
                        # drain: ONE DVE reduce per block-row (TTR pairing and
                        # gpsimd copies fault the device at full scale)
                        nc.vector.reduce_max(
                            maxacc[:, i * 8 + jx : i * 8 + jx + 1],
                            ps[:],
                            axis=mybir.AxisListType.X,
                        )
                    if j == 0 and "ttc" not in _KV:
                        # all slots of row-tile i done: finalize right away
                        nc.vector.reduce_max(
                            fin[:, i : i + 1],
                            maxacc[:, i * nslot : (i + 1) * nslot],
                            axis=mybir.AxisListType.X,
                        )
            nc.sync.dma_start(out=outd.rearrange("(i p) -> p i", p=P), in_=fin[:])

    nc.compile()
    return nc


_CACHE = {}


def _get_nc(N, D, NC):
    key = (N, D, NC)
    if key not in _CACHE:
        _CACHE[key] = _build(N, D, NC)
    return _CACHE[key]


def _host_stage(feats, NC):
    """Quantize to fp8 and build per-core inputs: raw fp8 shard + the
    all-gather result (normalized, scaled, fp8) rotated shard-first."""
    import ml_dtypes

    f8 = ml_dtypes.float8_e4m3
    N = feats.shape[0]
    SH = N // NC
    q = feats.astype(f8)
    qf = q.astype(np.float32)
    s = SCALE / np.sqrt((qf * qf).sum(axis=1, keepdims=True))
    xn = (qf * s).astype(f8)  # the all-gathered normalized features
    maps = []
    for c in range(NC):
        maps.append(
            {
                "gathered": np.ascontiguousarray(np.roll(xn, -c * SH, axis=0)),
                "shard": np.ascontiguousarray(q[c * SH : (c + 1) * SH]),
            }
        )
    return maps


def _loss_from_maxdot(m):
    maxcos = m.astype(np.float64) / (SCALE * SCALE)
    dist = np.sqrt(np.maximum(2.0 - 2.0 * maxcos, 0.0))
    return np.asarray(-np.mean(np.log(dist + 1e-8)), dtype=np.float32)


def kernel(features):
    from concourse.bass_utils import run_bass_kernel_spmd

    feats = np.asarray(features, dtype=np.float32)
    N, D = feats.shape
    nc = _get_nc(N, D, NCORES)
    res = run_bass_kernel_spmd(nc, _host_stage(feats, NCORES), list(range(NCORES)))
    m = np.concatenate([res.results[c]["maxdot"] for c in range(NCORES)])
    return _loss_from_maxdot(m)
